# revision 25
# baseline (speedup 1.0000x reference)
"""MoH (Mixture-of-Heads) attention kernel for Trainium2, 8 NeuronCores.

Strategy: data-parallel over batch (32 batches -> 4 per core), weights
replicated, no collectives. All matmuls in bf16 (fp32 PSUM accumulation).

Layout (per batch, per core):
  - host pre-transposes q/k/v to [B, D, S] (bf16) so projections need no
    on-device transposes; heads live in the partition dim of qpT/kpT
    (64 rows each), so transposed scores ST[k,q] = kh @ qh^T come straight
    from matmuls of qpT/kpT slices.
  - causal / partial masks are applied by PREFILLING the score PSUM with an
    additive mask via a tiny identity-matmul before the score matmul
    accumulates on top (PSUM zero-region semantics), so no vector op sits
    between the score matmul and the exp.
  - score k-blocks are packed into <=512-col PSUM tiles; one Exp per pack
    (no max-subtraction; scores are O(1)).  The reference's
    "scores row 0 := 0" rule becomes "STexp[:,0] := 1" plus ones-matmuls
    for the skipped k-blocks.
  - vp carries a ones-column so the attention matmul also produces the
    softmax denominator (row 64 of the [65, q] PSUM tile); normalization is
    reciprocal_approx_fast + gpsimd partition-broadcast + one fused DVE
    multiply that also applies the per-(batch,head) routing scalar.
  - routing: gate logits for all 4 s-blocks go into ONE PSUM bank
    ([128, 4*12] + a [1,12] mean-accumulator region); top-2-of-12 selection
    runs on raw exp values (monotone), and the 1/sum normalization is folded
    into the mean-matmul's stationary operand.
  - batches flow through a software pipeline: attention of batch i is
    interleaved (at head granularity) with the projection matmuls of batch
    i+1, so the tensor engine never drains while softmax chains retire.
"""

import sys

_TRN_REPO = "/opt/trn_rl_repo"
if _TRN_REPO not in sys.path:
    sys.path.insert(0, _TRN_REPO)

import numpy as np
import ml_dtypes

B, S, D = 32, 512, 1024
H, DK = 16, 64
H_SH, K_SEL = 4, 2
H_DYN = H - H_SH
N_CORES = 8
B_LOC = B // N_CORES
SB = S // 128      # 4 s-blocks
DT = D // 128      # 8 d-tiles
NEG = -1e9

_CACHE = {}
PROFILE = False          # set by test harness to capture an NTFF trace
LAST = {}                # exec_time_ns / profile path from the last run


def _classify_mask(mask):
    """Host-side: derive block structure from the [S,S] 0/1 mask.

    Returns (qs, mixed, uniq_tiles, causal) where
      qs[kb]    = first q (multiple of 128) kept for k-block kb, or None
      mixed[(qb,kb)] = index into uniq_tiles for blocks needing an
                  additive mask tile (maskT layout [k_local, q_local]),
                  or -1 for all-masked blocks inside the computed range
      uniq_tiles = list of [128,128] f32 additive tiles
      causal    = True if mask is exactly lower-triangular
    """
    m = mask.astype(bool)
    if not m[1:].any(axis=1).all():
        raise NotImplementedError(
            "a query row (>0) is fully masked; uniform-softmax fallback "
            "for fully-masked rows is not implemented"
        )
    causal = bool(np.array_equal(m, np.tril(np.ones_like(m))))
    qs = []
    mixed = {}
    uniq = []
    uniq_key = {}
    for kb in range(SB):
        first = None
        for qb in range(SB):
            blk = m[qb * 128:(qb + 1) * 128, kb * 128:(kb + 1) * 128]
            if blk.any():
                if first is None:
                    first = qb * 128
                if not blk.all():
                    add = np.where(blk.T, 0.0, np.float32(NEG)).astype(np.float32)
                    key = add.tobytes()
                    if key not in uniq_key:
                        uniq_key[key] = len(uniq)
                        uniq.append(add)
                    mixed[(qb, kb)] = uniq_key[key]
            elif first is not None:
                mixed[(qb, kb)] = -1
        qs.append(first)
    return qs, mixed, uniq, causal


def _make_packs(qs):
    """Pack the per-k-block score ranges into <=512-col PSUM tiles.

    Returns (packs, se_cols) where packs is a list of groups, each group a
    list of (kb, q0, n, off_in_tile, se_col) and se_cols the total width of
    the per-head exp output tile.
    """
    packs = []
    cur, used = [], 0
    se_col = 0
    for kb in range(SB):
        if qs[kb] is None:
            continue
        n = S - qs[kb]
        if used + n > 512 and cur:
            packs.append(cur)
            cur, used = [], 0
        cur.append((kb, qs[kb], n, used, se_col))
        used += n
        se_col += n
    if cur:
        packs.append(cur)
    return packs, se_col


def _build(qs, mixed, uniq_n, causal, b_loc=B_LOC, has_bvo=False, repeat=1):
    import os
    ST_BUFS = int(os.environ.get("K_ST_BUFS", "3"))
    MM_BUFS = int(os.environ.get("K_MM_BUFS", "2"))
    CTX_BUFS = int(os.environ.get("K_CTX_BUFS", "3"))
    SE_BUFS = int(os.environ.get("K_SE_BUFS", "4"))
    IN_BUFS = int(os.environ.get("K_IN_BUFS", "2"))
    # split score matmuls at prefilled/pristine boundaries: required for the
    # CoreSim interpreter (all-or-nothing pending-zero assert); on HW the
    # PSUM zero-region is per-byte so the unsplit form is also valid.
    SPLIT = int(os.environ.get("K_SPLIT", "1"))
    NORM = os.environ.get("K_NORM", "recip")  # "recip" | "divide"
    # causal masks: zero the upper triangle of the transposed diagonal
    # blocks post-exp on the (otherwise idle) GPSIMD engine instead of
    # prefilling additive masks through the PE
    MASKMODE = os.environ.get("K_MASKMODE", "affine" if causal else "prefill")
    CTXDELAY = int(os.environ.get("K_CTXDELAY", "1"))
    import concourse.bacc as bacc
    import concourse.tile as tile
    import concourse.mybir as mybir
    from contextlib import ExitStack

    f32 = mybir.dt.float32
    bf16 = mybir.dt.bfloat16
    AF = mybir.ActivationFunctionType
    ALU = mybir.AluOpType

    nc = bacc.Bacc(trn_type="TRN2", target_bir_lowering=False, debug=False)

    qT = nc.dram_tensor("qT", [b_loc, D, S], bf16, kind="ExternalInput").ap()
    kT = nc.dram_tensor("kT", [b_loc, D, S], bf16, kind="ExternalInput").ap()
    vT = nc.dram_tensor("vT", [b_loc, D, S], bf16, kind="ExternalInput").ap()
    wq = nc.dram_tensor("wq", [D, D], bf16, kind="ExternalInput").ap()
    wk = nc.dram_tensor("wk", [D, D], bf16, kind="ExternalInput").ap()
    wv = nc.dram_tensor("wv", [D, D], bf16, kind="ExternalInput").ap()
    wo = nc.dram_tensor("wo", [D, D], bf16, kind="ExternalInput").ap()
    wg = nc.dram_tensor("wg", [D, H_DYN], bf16, kind="ExternalInput").ap()
    bqt = nc.dram_tensor("bqt", [128, DT], f32, kind="ExternalInput").ap()
    bkt = nc.dram_tensor("bkt", [128, DT], f32, kind="ExternalInput").ap()
    ident = nc.dram_tensor("ident", [128, 128], bf16, kind="ExternalInput").ap()
    if has_bvo:
        bvb = nc.dram_tensor("bvb", [1, D], f32, kind="ExternalInput").ap()
        bob = nc.dram_tensor("bob", [1, D], f32, kind="ExternalInput").ap()
    if uniq_n:
        maskt = nc.dram_tensor(
            "maskt", [uniq_n, 128, 128], bf16, kind="ExternalInput"
        ).ap()
    out = nc.dram_tensor("out", [b_loc, S, D], f32, kind="ExternalOutput").ap()
    DEBUG = int(os.environ.get("K_DEBUG", "0"))
    if DEBUG:
        dbg = {}
        for nm, shape, dt_ in (
            ("d_qpT", [128, DT, S], bf16), ("d_kpT", [128, DT, S], bf16),
            ("d_vp", [128, SB, H, DK + 1], bf16), ("d_se", [128, 1280], bf16),
            ("d_bc", [64, S], f32), ("d_routing", [128, H], f32),
            ("d_ctxT", [128, DT, S], bf16), ("d_ctxps", [DK + 1, S], f32),
            ("d_recip", [1, S], f32),
        ):
            dbg[nm] = nc.dram_tensor(nm, shape, dt_, kind="ExternalOutput").ap()

    packs, se_cols = _make_packs(qs)
    need_allneg = any(v == -1 for v in mixed.values())

    with tile.TileContext(nc) as tc, ExitStack() as ctx:
        const = ctx.enter_context(tc.tile_pool(name="const", bufs=1))
        act = ctx.enter_context(tc.tile_pool(name="act", bufs=2))
        small = ctx.enter_context(tc.tile_pool(name="small", bufs=2))
        psum = ctx.enter_context(tc.tile_pool(name="psum", bufs=1, space="PSUM"))

        def load_inputs(b):
            ins = {}
            for nm, ap in (("q", qT), ("k", kT), ("v", vT)):
                # v is consumed by the last projection tasks of a stage, so a
                # single buffer stalls nothing while saving 8KB/partition
                t = act.tile([128, DT, S], bf16, name=f"in_{nm}", tag=f"in_{nm}",
                             bufs=1 if nm == "v" else IN_BUFS)
                nc.sync.dma_start(
                    t[:, :, :], ap[b].rearrange("(t p) s -> p t s", p=128))
                ins[nm] = t
            return ins

        # ---- pair-0 inputs first so projections can start immediately --
        pending_ins = {0: load_inputs(0)}

        # ---- constants (wq/wk/wv early, wo last) -----------------------
        w_tiles = {}
        for wname, wap in (("wq", wq), ("wk", wk), ("wv", wv), ("wo", wo)):
            t = const.tile([128, DT, D], bf16, name=wname, tag=wname)
            if wname != "wo":
                nc.sync.dma_start(
                    t[:, :, :], wap.rearrange("(t p) o -> p t o", p=128))
            w_tiles[wname] = t
        if b_loc > 1:
            pending_ins[1] = load_inputs(1)
        wg_sb = const.tile([128, DT, H_DYN], bf16, name="wg_sb", tag="wg_sb")
        nc.sync.dma_start(wg_sb[:, :, :], wg.rearrange("(t p) j -> p t j", p=128))
        bq_sb = const.tile([128, DT], f32, name="bq_sb", tag="bq_sb")
        nc.sync.dma_start(bq_sb[:], bqt[:])
        bk_sb = const.tile([128, DT], f32, name="bk_sb", tag="bk_sb")
        nc.sync.dma_start(bk_sb[:], bkt[:])
        ident_sb = const.tile([128, 128], bf16, name="ident_sb", tag="ident_sb")
        nc.sync.dma_start(ident_sb[:], ident[:])
        if has_bvo:
            bv_sb = const.tile([1, D], f32, name="bv_sb", tag="bv_sb")
            nc.sync.dma_start(bv_sb[:], bvb[:])
            bo_sb = const.tile([1, D], f32, name="bo_sb", tag="bo_sb")
            nc.sync.dma_start(bo_sb[:], bob[:])
            bvb_sb = const.tile([128, D], f32, name="bvb_sb", tag="bvb_sb")
            nc.gpsimd.partition_broadcast(bvb_sb[:], bv_sb[:])
            bob_sb = const.tile([128, D], f32, name="bob_sb", tag="bob_sb")
            nc.gpsimd.partition_broadcast(bob_sb[:], bo_sb[:])

        mask_tiles = []
        for u in range(uniq_n):
            t = const.tile([128, 128], bf16, name=f"mask{u}", tag=f"mask{u}")
            nc.sync.dma_start(t[:], maskt[u])
            mask_tiles.append(t)
        if need_allneg:
            allneg = const.tile([128, 128], bf16, name="allneg", tag="allneg")
            nc.vector.memset(allneg[:], NEG)

        ones_bf = const.tile([128, 1], bf16, name="ones_bf", tag="ones_bf")
        nc.vector.memset(ones_bf[:], 1.0)
        nc.sync.dma_start(
            w_tiles["wo"][:, :, :], wo.rearrange("(t p) o -> p t o", p=128))

        # ---------------------------------------------------------------
        def proj_tasks(b, ins, st):
            """Return a list of closures emitting the projection matmul
            groups + routing for batch b; st is this batch's state dict."""
            tasks = []

            def qk_group(dst, src, wn, bias, eng, t):
                def run():
                    ps = psum.tile([128, S], f32, name="mm_ps", tag="mm",
                                   bufs=MM_BUFS)
                    for d in range(DT):
                        nc.tensor.matmul(
                            ps[:],
                            w_tiles[wn][:, d, t * 128:(t + 1) * 128],
                            src[:, d, :],
                            start=(d == 0),
                            stop=(d == DT - 1),
                        )
                    if eng == "act":
                        nc.scalar.activation(
                            dst[:, t, :], ps[:], AF.Identity,
                            bias=bias[:, t:t + 1],
                        )
                    else:
                        nc.vector.tensor_scalar_add(
                            dst[:, t, :], ps[:], bias[:, t:t + 1]
                        )
                return run

            for t in range(DT):
                tasks.append(qk_group(st["qpT"], ins["q"], "wq", bq_sb, "act", t))
            for t in range(DT):
                tasks.append(qk_group(st["kpT"], ins["k"], "wk", bk_sb, "dve", t))

            def vp_group(sb, c):
                def run():
                    vp = st["vp"]
                    ps = psum.tile([128, S], f32, name="mm_ps", tag="mm",
                                   bufs=MM_BUFS)
                    for d in range(DT):
                        nc.tensor.matmul(
                            ps[:],
                            ins["v"][:, d, sb * 128:(sb + 1) * 128],
                            w_tiles["wv"][:, d, c * 512:(c + 1) * 512],
                            start=(d == 0),
                            stop=(d == DT - 1),
                        )
                    src2 = ps[:].rearrange("p (h e) -> p h e", e=DK)
                    dst2 = vp[:, sb, c * 8:(c + 1) * 8, 0:DK]
                    if has_bvo:
                        nc.vector.scalar_tensor_tensor(
                            dst2, src2, 1.0,
                            bvb_sb[:, c * 512:(c + 1) * 512].rearrange(
                                "p (h e) -> p h e", e=DK),
                            op0=ALU.mult, op1=ALU.add,
                        )
                    else:
                        nc.vector.tensor_copy(dst2, src2)
                return run

            for sb in range(SB):
                for c in range(2):
                    tasks.append(vp_group(sb, c))

            def routing_task():
                qpT = st["qpT"]
                # one PSUM bank (shared with the score tiles' rotation):
                # cols 0:48 = gate logits (4 sb x 12), cols 48:60 = routing
                # mean accumulator ([1,12] region)
                ps_g = psum.tile([128, 512], f32, name="ps_g", tag="st",
                                 bufs=ST_BUFS)
                for sb in range(SB):
                    for t in range(DT):
                        nc.tensor.matmul(
                            ps_g[:, sb * H_DYN:(sb + 1) * H_DYN],
                            qpT[:, t, sb * 128:(sb + 1) * 128],
                            wg_sb[:, t, :],
                            start=(sb == 0 and t == 0),
                            stop=False,
                            skip_group_check=True,
                        )
                gexp = small.tile([128, SB, H_DYN], f32, name="gexp", tag="gexp")
                nc.scalar.activation(
                    gexp[:], ps_g[:, 0:SB * H_DYN].rearrange(
                        "p (s j) -> p s j", j=H_DYN),
                    AF.Exp)
                gsum = small.tile([128, SB], f32, name="gsum", tag="gsum")
                nc.vector.tensor_reduce(
                    gsum[:], gexp[:], axis=mybir.AxisListType.X, op=ALU.add)
                ginv = small.tile([128, SB], f32, name="ginv", tag="ginv")
                nc.vector.reciprocal_approx_fast(ginv[:], gsum[:])
                m1 = small.tile([128, SB], f32, name="m1", tag="m1")
                nc.vector.tensor_reduce(
                    m1[:], gexp[:], axis=mybir.AxisListType.X, op=ALU.max)
                g2 = small.tile([128, SB, H_DYN], f32, name="g2", tag="g2")
                m2 = small.tile([128, SB], f32, name="m2", tag="m2")
                sel = small.tile([128, SB, H_DYN], f32, name="sel", tag="sel")
                for sb in range(SB):
                    # knock out the top-1, re-max for top-2 threshold
                    eqm = small.tile([128, H_DYN], f32, name="eqm", tag="eqm")
                    nc.vector.tensor_scalar(
                        eqm[:], gexp[:, sb, :], m1[:, sb:sb + 1], None,
                        op0=ALU.is_equal)
                    nc.vector.scalar_tensor_tensor(
                        g2[:, sb, :], eqm[:], NEG, gexp[:, sb, :],
                        op0=ALU.mult, op1=ALU.add)
                nc.vector.tensor_reduce(
                    m2[:], g2[:], axis=mybir.AxisListType.X, op=ALU.max)
                for sb in range(SB):
                    nc.vector.tensor_scalar(
                        sel[:, sb, :], gexp[:, sb, :], m2[:, sb:sb + 1], None,
                        op0=ALU.is_ge)
                gdyn = small.tile([128, SB, H_DYN], f32, name="gdyn", tag="gdyn")
                nc.vector.tensor_tensor(gdyn[:], gexp[:], sel[:], op=ALU.mult)
                for sb in range(SB):
                    nc.tensor.matmul(
                        ps_g[0:1, 48:48 + H_DYN],
                        ginv[:, sb:sb + 1],
                        gdyn[:, sb, :],
                        start=False, stop=(sb == SB - 1),
                        skip_group_check=True,
                    )
                routing_sb = small.tile([1, H], f32, name="routing_sb",
                                        tag="routing_sb")
                nc.vector.memset(routing_sb[0:1, 0:H_SH], 1.0)
                nc.scalar.mul(routing_sb[0:1, H_SH:H], ps_g[0:1, 48:48 + H_DYN],
                              1.0 / S)
                routing_bc = small.tile([128, H], f32, name="routing_bc",
                                        tag="routing_bc")
                nc.gpsimd.partition_broadcast(routing_bc[:], routing_sb[:])
                st["routing"] = routing_bc

            tasks.append(routing_task)
            return tasks

        def scores_phase(st, h):
            """Emit prefills + score matmuls + exp for head h; return the
            deferred ctx matmul operand list."""
            qpT, kpT, vp = st["qpT"], st["kpT"], st["vp"]
            se = act.tile([128, se_cols], bf16, name="se", tag="se",
                          bufs=SE_BUFS)
            ctx_mms = []
            ph = (h % 2) * 64
            th = h // 2
            for grp in packs:
                tot = grp[-1][3] + grp[-1][2]
                ps_st = psum.tile([128, 512], f32, name="ps_st", tag="st",
                                  bufs=ST_BUFS)
                first = True
                for gi, (kb, q0, n, off, sc) in enumerate(grp):
                    if MASKMODE == "prefill":
                        # additive-mask prefill: tiny identity matmuls drop
                        # the mask into PSUM; the score matmul accumulates
                        # on top.
                        for qb in range(q0 // 128, SB):
                            mi = mixed.get((qb, kb))
                            if mi is None:
                                continue
                            col = off + qb * 128 - q0
                            src = mask_tiles[mi] if mi >= 0 else allneg
                            nc.tensor.matmul(
                                ps_st[:, col:col + 128], ident_sb[:], src[:],
                                start=first, stop=False, skip_group_check=True,
                            )
                            first = False
                    # score matmul, optionally split at prefilled/pristine
                    # block boundaries (uniform PSUM regions for CoreSim)
                    if SPLIT and MASKMODE == "prefill":
                        runs = []
                        for qb in range(q0 // 128, SB):
                            pref = (qb, kb) in mixed
                            if runs and runs[-1][2] == pref:
                                runs[-1][1] = (qb + 1) * 128
                            else:
                                runs.append([qb * 128, (qb + 1) * 128, pref])
                    else:
                        runs = [[q0, S, False]]
                    last_entry = gi == len(grp) - 1
                    for ri, (rs, re, pref) in enumerate(runs):
                        nc.tensor.matmul(
                            ps_st[:, off + rs - q0:off + re - q0],
                            kpT[ph:ph + 64, th, kb * 128:(kb + 1) * 128],
                            qpT[ph:ph + 64, th, rs:re],
                            start=first,
                            stop=(last_entry and ri == len(runs) - 1),
                            skip_group_check=True,
                        )
                        first = False
                    ctx_mms.append((vp[:, kb, h, :], se[:, sc:sc + n],
                                    (q0, n)))
                sc0 = grp[0][4]
                nc.scalar.activation(
                    se[:, sc0:sc0 + tot], ps_st[:, 0:tot], AF.Exp,
                    scale=1.0 / np.sqrt(DK))
                if MASKMODE == "affine":
                    # zero se where q_local < k_local on the transposed
                    # diagonal blocks (iota = q_local - k_local)
                    for (kb, q0, n, off, sc) in grp:
                        for qb in range(q0 // 128, SB):
                            if (qb, kb) in mixed:
                                col = sc + qb * 128 - q0
                                nc.gpsimd.affine_select(
                                    se[:, col:col + 128],
                                    se[:, col:col + 128],
                                    pattern=[[1, 128]],
                                    compare_op=ALU.is_ge,
                                    fill=0.0,
                                    base=0,
                                    channel_multiplier=-1,
                                )
                for (kb, q0, n, off, sc) in grp:
                    if q0 == 0:
                        nc.vector.memset(se[:, sc:sc + 1], 1.0)
            if DEBUG and dbg_flag.get("se"):
                dbg_flag["se"] = False
                nc.sync.dma_start(dbg["d_se"][:, 0:se_cols], se[:])
            return ctx_mms

        def ctx_phase(st, h, ctx_mms):
            vp, ctxT = st["vp"], st["ctxT"]
            ph = (h % 2) * 64
            th = h // 2
            ps_ctx = psum.tile([DK + 1, S], f32, name="ps_ctx", tag="ctx",
                               bufs=CTX_BUFS)
            # ascending q0 so the first (widest) matmul covers all later
            # ones' columns — keeps each PSUM region uniformly pending
            ctx_mms.sort(key=lambda m: m[2][0])
            mms = [(l, r, ps_ctx[:, q0:q0 + n]) for (l, r, (q0, n)) in ctx_mms]
            for kb in range(SB):
                # every k-block whose score matmul does not cover q=0 must
                # still contribute there: the reference zeroes score row 0
                # for ALL k (incl. fully-masked blocks), making exp()=1.
                if qs[kb] != 0:
                    mms.append((vp[:, kb, h, :], ones_bf[:], ps_ctx[:, 0:1]))
            for i, (lhsT, rhs, dst) in enumerate(mms):
                nc.tensor.matmul(
                    dst, lhsT, rhs,
                    start=(i == 0), stop=(i == len(mms) - 1),
                    skip_group_check=True,
                )

            # denominator row -> SBUF via ACT (a standard op with correct
            # PSUM-drain hazard handling; the custom-DVE reciprocal reading
            # PSUM directly raced the PE writeback on HW)
            drow = small.tile([1, S], f32, name="drow", tag="drow", bufs=2)
            nc.scalar.copy(drow[:], ps_ctx[DK:DK + 1, :])
            bc = small.tile([64, S], f32, name="bc", tag="bc", bufs=2)
            if NORM == "divide":
                nc.gpsimd.partition_broadcast(bc[:], drow[:], channels=64)
                op1 = ALU.divide
            else:
                recip = small.tile([1, S], f32, name="recip", tag="recip",
                                   bufs=2)
                nc.vector.reciprocal_approx_fast(recip[:], drow[:])
                nc.gpsimd.partition_broadcast(bc[:], recip[:], channels=64)
                op1 = ALU.mult
            if DEBUG and dbg_flag.get("bc"):
                dbg_flag["bc"] = False
                dps = small.tile([DK + 1, S], f32, name="dps", tag="dps")
                nc.scalar.copy(dps[:], ps_ctx[:])
                nc.sync.dma_start(dbg["d_ctxps"][:], dps[:])
                nc.sync.dma_start(dbg["d_recip"][:], drow[:])
                nc.sync.dma_start(dbg["d_bc"][:], bc[:])
            nc.vector.scalar_tensor_tensor(
                ctxT[ph:ph + 64, th, :],
                ps_ctx[0:DK, :],
                st["routing"][0:64, h:h + 1],
                bc[:],
                op0=ALU.mult, op1=op1,
            )

        def outproj_tasks(st, b):
            ctxT = st["ctxT"]

            def group(sb, c):
                def run():
                    ps = psum.tile([128, S], f32, name="mm_ps", tag="mm",
                                   bufs=MM_BUFS)
                    for t in range(DT):
                        nc.tensor.matmul(
                            ps[:],
                            ctxT[:, t, sb * 128:(sb + 1) * 128],
                            w_tiles["wo"][:, t, c * 512:(c + 1) * 512],
                            start=(t == 0),
                            stop=(t == DT - 1),
                        )
                    ob = small.tile([128, S], f32, name="ob", tag="ob", bufs=2)
                    if has_bvo:
                        nc.vector.scalar_tensor_tensor(
                            ob[:], ps[:], 1.0, bob_sb[:, c * 512:(c + 1) * 512],
                            op0=ALU.mult, op1=ALU.add,
                        )
                    else:
                        nc.scalar.copy(ob[:], ps[:])
                    nc.sync.dma_start(
                        out[b, sb * 128:(sb + 1) * 128, c * 512:(c + 1) * 512],
                        ob[:],
                    )
                return run

            return [group(sb, c) for sb in range(SB) for c in range(2)]

        def merge_tasks(a, bl):
            """Interleave two task lists by fractional position."""
            keyed = [((j + 0.5) / max(len(a), 1), t) for j, t in enumerate(a)]
            keyed += [((j + 0.5) / max(len(bl), 1), t) for j, t in enumerate(bl)]
            keyed.sort(key=lambda kt: kt[0])
            return [t for _, t in keyed]

        def new_state():
            return {
                "qpT": act.tile([128, DT, S], bf16, name="qpT", tag="qpT"),
                "kpT": act.tile([128, DT, S], bf16, name="kpT", tag="kpT"),
                "vp": None,
                "ctxT": act.tile([128, DT, S], bf16, name="ctxT", tag="ctxT"),
                "routing": None,
            }

        def make_vp(st):
            vp = act.tile([128, SB, H, DK + 1], bf16, name="vp", tag="vp")
            nc.vector.memset(vp[:, :, :, DK:DK + 1], 1.0)
            st["vp"] = vp

        # ---- software pipeline: attn(b) || proj(b+1) -------------------
        order = [bb for _ in range(repeat) for bb in range(b_loc)]
        first = True
        dbg_flag = {"se": DEBUG, "bc": DEBUG}

        # prologue: projections of the first batch
        st_cur = new_state()
        make_vp(st_cur)
        ins0 = pending_ins.pop(order[0], None) or load_inputs(order[0])
        for t in proj_tasks(order[0], ins0, st_cur):
            t()
        if DEBUG:
            nc.sync.dma_start(dbg["d_qpT"][:, :, :], st_cur["qpT"][:, :, :])
            nc.sync.dma_start(dbg["d_kpT"][:, :, :], st_cur["kpT"][:, :, :])
            nc.sync.dma_start(dbg["d_vp"][:, :, :, :], st_cur["vp"][:, :, :, :])
            nc.sync.dma_start(dbg["d_routing"][:], st_cur["routing"][:])

        st_prev = None
        b_prev = None
        for i, b in enumerate(order):
            nxt = order[i + 1] if i + 1 < len(order) else None
            tasks = []
            st_nxt = None
            if nxt is not None:
                st_nxt = new_state()
                make_vp(st_nxt)
                if first and nxt in pending_ins:
                    ins = pending_ins.pop(nxt)
                else:
                    ins = load_inputs(nxt)
                tasks = proj_tasks(nxt, ins, st_nxt)
            first = False
            # out-projection of the PREVIOUS batch rides in this stage's
            # task interleave so it never waits on the last heads' chains
            if st_prev is not None:
                tasks = merge_tasks(outproj_tasks(st_prev, b_prev), tasks)
            ntasks = len(tasks)
            done = 0
            pend = []
            for h in range(H):
                cm = scores_phase(st_cur, h)
                if len(pend) >= CTXDELAY:
                    ctx_phase(st_cur, *pend.pop(0))
                pend.append((h, cm))
                want = (h + 1) * ntasks // H
                while done < want:
                    tasks[done]()
                    done += 1
            for p in pend:
                ctx_phase(st_cur, *p)
            if DEBUG and i == 0:
                nc.sync.dma_start(dbg["d_ctxT"][:, :, :], st_cur["ctxT"][:, :, :])
            st_prev, b_prev = st_cur, b
            st_cur = st_nxt
        for t in outproj_tasks(st_prev, b_prev):
            t()

    nc.compile()
    return nc


def _host_prep(inputs):
    """Host-side input prep shared by kernel() and the timing harness.

    Returns (shared, qT, kT, vT, cache_key_parts).
    """
    q = np.asarray(inputs["q"])
    k = np.asarray(inputs["k"])
    v = np.asarray(inputs["v"])
    mask = np.asarray(inputs["mask"]).reshape(S, S)
    Wq, bq = np.asarray(inputs["Wq"]), np.asarray(inputs["bq"])
    Wk, bk = np.asarray(inputs["Wk"]), np.asarray(inputs["bk"])
    Wv, bv = np.asarray(inputs["Wv"]), np.asarray(inputs["bv"])
    Wg = np.asarray(inputs["Wg"])
    Wo, bo = np.asarray(inputs["Wo"]), np.asarray(inputs["bo"])

    bf = ml_dtypes.bfloat16
    qs, mixed, uniq, causal = _classify_mask(mask)
    has_bvo = bool(np.any(bv) or np.any(bo))

    qT = np.ascontiguousarray(q.astype(bf).transpose(0, 2, 1))
    kT = np.ascontiguousarray(k.astype(bf).transpose(0, 2, 1))
    vT = np.ascontiguousarray(v.astype(bf).transpose(0, 2, 1))

    shared = {
        "wq": Wq.astype(bf), "wk": Wk.astype(bf), "wv": Wv.astype(bf),
        "wo": Wo.astype(bf), "wg": Wg.astype(bf),
        "bqt": np.ascontiguousarray(
            bq.astype(np.float32).reshape(DT, 128).T),
        "bkt": np.ascontiguousarray(
            bk.astype(np.float32).reshape(DT, 128).T),
        "ident": np.eye(128, dtype=bf),
    }
    if has_bvo:
        shared["bvb"] = bv.astype(np.float32).reshape(1, D)
        shared["bob"] = bo.astype(np.float32).reshape(1, D)
    if uniq:
        shared["maskt"] = np.stack(uniq, axis=0).astype(bf)
    return shared, qT, kT, vT, (mask, qs, mixed, uniq, causal, has_bvo)


def kernel(**inputs):
    shared, qT, kT, vT, (mask, qs, mixed, uniq, causal, has_bvo) = \
        _host_prep(inputs)

    cache_key = ("v3", mask.tobytes(), has_bvo)
    if cache_key not in _CACHE:
        _CACHE[cache_key] = _build(qs, mixed, len(uniq), causal, has_bvo=has_bvo)
    nc = _CACHE[cache_key]

    in_maps = []
    for c in range(N_CORES):
        sl = slice(c * B_LOC, (c + 1) * B_LOC)
        m = dict(shared)
        m["qT"] = qT[sl]
        m["kT"] = kT[sl]
        m["vT"] = vT[sl]
        in_maps.append(m)

    from concourse.bass_utils import run_bass_kernel_spmd

    kw = {}
    if PROFILE:
        import tempfile
        kw = dict(trace=True, tmpdir=tempfile.mkdtemp(prefix="moh_trace_"))
    res = None
    last_exc = None
    for _attempt in range(3):
        try:
            res = run_bass_kernel_spmd(
                nc, in_maps, core_ids=list(range(N_CORES)), **kw)
            break
        except Exception as e:  # transient axon/NRT device errors
            last_exc = e
    if res is None:
        raise last_exc
    LAST["exec_time_ns"] = res.exec_time_ns
    LAST["profile_json"] = res.profile_json
    if PROFILE:
        LAST["tmpdir"] = kw.get("tmpdir")
    outs = [res.results[c]["out"] for c in range(N_CORES)]
    return np.concatenate(outs, axis=0).astype(np.float32)


# revision 31
# speedup vs baseline: 1.4730x; 1.4730x over previous
"""MoH (Mixture-of-Heads) attention kernel for Trainium2, 8 NeuronCores.

Strategy: data-parallel over batch (32 batches -> 4 per core), weights
replicated, no collectives. All matmuls in bf16 (fp32 PSUM accumulation).

Layout (per batch, per core):
  - host pre-transposes q/k/v to [B, D, S] (bf16) so projections need no
    on-device transposes; heads live in the partition dim of qpT/kpT
    (64 rows each), so transposed scores ST[k,q] = kh @ qh^T come straight
    from matmuls of qpT/kpT slices.
  - causal / partial masks are applied by PREFILLING the score PSUM with an
    additive mask via a tiny identity-matmul before the score matmul
    accumulates on top (PSUM zero-region semantics), so no vector op sits
    between the score matmul and the exp.
  - score k-blocks are packed into <=512-col PSUM tiles; one Exp per pack
    (no max-subtraction; scores are O(1)).  The reference's
    "scores row 0 := 0" rule becomes "STexp[:,0] := 1" plus ones-matmuls
    for the skipped k-blocks.
  - vp carries a ones-column so the attention matmul also produces the
    softmax denominator (row 64 of the [65, q] PSUM tile); normalization is
    reciprocal_approx_fast + gpsimd partition-broadcast + one fused DVE
    multiply that also applies the per-(batch,head) routing scalar.
  - routing: gate logits for all 4 s-blocks go into ONE PSUM bank
    ([128, 4*12] + a [1,12] mean-accumulator region); top-2-of-12 selection
    runs on raw exp values (monotone), and the 1/sum normalization is folded
    into the mean-matmul's stationary operand.
  - batches flow through a software pipeline: attention of batch i is
    interleaved (at head granularity) with the projection matmuls of batch
    i+1, so the tensor engine never drains while softmax chains retire.
"""

import sys

_TRN_REPO = "/opt/trn_rl_repo"
if _TRN_REPO not in sys.path:
    sys.path.insert(0, _TRN_REPO)

import numpy as np
import ml_dtypes

B, S, D = 32, 512, 1024
H, DK = 16, 64
H_SH, K_SEL = 4, 2
H_DYN = H - H_SH
N_CORES = 8
B_LOC = B // N_CORES
SB = S // 128      # 4 s-blocks
DT = D // 128      # 8 d-tiles
NEG = -1e9

_CACHE = {}
PROFILE = False          # set by test harness to capture an NTFF trace
LAST = {}                # exec_time_ns / profile path from the last run


def _classify_mask(mask):
    """Host-side: derive block structure from the [S,S] 0/1 mask.

    Returns (qs, mixed, uniq_tiles, causal) where
      qs[kb]    = first q (multiple of 128) kept for k-block kb, or None
      mixed[(qb,kb)] = index into uniq_tiles for blocks needing an
                  additive mask tile (maskT layout [k_local, q_local]),
                  or -1 for all-masked blocks inside the computed range
      uniq_tiles = list of [128,128] f32 additive tiles
      causal    = True if mask is exactly lower-triangular
    """
    m = mask.astype(bool)
    if not m[1:].any(axis=1).all():
        raise NotImplementedError(
            "a query row (>0) is fully masked; uniform-softmax fallback "
            "for fully-masked rows is not implemented"
        )
    causal = bool(np.array_equal(m, np.tril(np.ones_like(m))))
    qs = []
    mixed = {}
    uniq = []
    uniq_key = {}
    for kb in range(SB):
        first = None
        for qb in range(SB):
            blk = m[qb * 128:(qb + 1) * 128, kb * 128:(kb + 1) * 128]
            if blk.any():
                if first is None:
                    first = qb * 128
                if not blk.all():
                    add = np.where(blk.T, 0.0, np.float32(NEG)).astype(np.float32)
                    key = add.tobytes()
                    if key not in uniq_key:
                        uniq_key[key] = len(uniq)
                        uniq.append(add)
                    mixed[(qb, kb)] = uniq_key[key]
            elif first is not None:
                mixed[(qb, kb)] = -1
        qs.append(first)
    return qs, mixed, uniq, causal


def _make_packs(qs):
    """Pack the per-k-block score ranges into <=512-col PSUM tiles.

    Returns (packs, se_cols) where packs is a list of groups, each group a
    list of (kb, q0, n, off_in_tile, se_col) and se_cols the total width of
    the per-head exp output tile.
    """
    packs = []
    cur, used = [], 0
    se_col = 0
    for kb in range(SB):
        if qs[kb] is None:
            continue
        n = S - qs[kb]
        if used + n > 512 and cur:
            packs.append(cur)
            cur, used = [], 0
        cur.append((kb, qs[kb], n, used, se_col))
        used += n
        se_col += n
    if cur:
        packs.append(cur)
    return packs, se_col


def _build(qs, mixed, uniq_n, causal, b_loc=B_LOC, has_bvo=False, repeat=1):
    import os
    ST_BUFS = int(os.environ.get("K_ST_BUFS", "3"))
    MM_BUFS = int(os.environ.get("K_MM_BUFS", "2"))
    CTX_BUFS = int(os.environ.get("K_CTX_BUFS", "3"))
    SE_BUFS = int(os.environ.get("K_SE_BUFS", "4"))
    IN_BUFS = int(os.environ.get("K_IN_BUFS", "2"))
    # split score matmuls at prefilled/pristine boundaries: required for the
    # CoreSim interpreter (all-or-nothing pending-zero assert); on HW the
    # PSUM zero-region is per-byte so the unsplit form is also valid.
    SPLIT = int(os.environ.get("K_SPLIT", "1"))
    NORM = os.environ.get("K_NORM", "recip")  # "recip" | "divide"
    # causal masks: zero the upper triangle of the transposed diagonal
    # blocks post-exp on the (otherwise idle) GPSIMD engine instead of
    # prefilling additive masks through the PE
    MASKMODE = os.environ.get("K_MASKMODE", "affine" if causal else "prefill")
    CTXDELAY = int(os.environ.get("K_CTXDELAY", "1"))
    import concourse.bacc as bacc
    import concourse.tile as tile
    import concourse.mybir as mybir
    from contextlib import ExitStack

    f32 = mybir.dt.float32
    bf16 = mybir.dt.bfloat16
    AF = mybir.ActivationFunctionType
    ALU = mybir.AluOpType

    nc = bacc.Bacc(trn_type="TRN2", target_bir_lowering=False, debug=False)

    qT = nc.dram_tensor("qT", [b_loc, D, S], bf16, kind="ExternalInput").ap()
    kT = nc.dram_tensor("kT", [b_loc, D, S], bf16, kind="ExternalInput").ap()
    vT = nc.dram_tensor("vT", [b_loc, D, S], bf16, kind="ExternalInput").ap()
    wq = nc.dram_tensor("wq", [D, D], bf16, kind="ExternalInput").ap()
    wk = nc.dram_tensor("wk", [D, D], bf16, kind="ExternalInput").ap()
    wv = nc.dram_tensor("wv", [D, D], bf16, kind="ExternalInput").ap()
    wo = nc.dram_tensor("wo", [D, D], bf16, kind="ExternalInput").ap()
    wg = nc.dram_tensor("wg", [D, H_DYN], bf16, kind="ExternalInput").ap()
    bqt = nc.dram_tensor("bqt", [128, DT], f32, kind="ExternalInput").ap()
    bkt = nc.dram_tensor("bkt", [128, DT], f32, kind="ExternalInput").ap()
    ident = nc.dram_tensor("ident", [128, 128], bf16, kind="ExternalInput").ap()
    if has_bvo:
        bvb = nc.dram_tensor("bvb", [1, D], f32, kind="ExternalInput").ap()
        bob = nc.dram_tensor("bob", [1, D], f32, kind="ExternalInput").ap()
    if uniq_n:
        maskt = nc.dram_tensor(
            "maskt", [uniq_n, 128, 128], bf16, kind="ExternalInput"
        ).ap()
    out = nc.dram_tensor("out", [b_loc, S, D], f32, kind="ExternalOutput").ap()
    DEBUG = int(os.environ.get("K_DEBUG", "0"))
    if DEBUG:
        dbg = {}
        for nm, shape, dt_ in (
            ("d_qpT", [128, DT, S], bf16), ("d_kpT", [128, DT, S], bf16),
            ("d_vp", [128, SB, H, DK + 1], bf16), ("d_se", [128, 1280], bf16),
            ("d_bc", [64, S], f32), ("d_routing", [128, H], f32),
            ("d_ctxT", [128, DT, S], bf16), ("d_ctxps", [DK + 1, S], f32),
            ("d_recip", [1, S], f32),
        ):
            dbg[nm] = nc.dram_tensor(nm, shape, dt_, kind="ExternalOutput").ap()

    packs, se_cols = _make_packs(qs)
    need_allneg = any(v == -1 for v in mixed.values())

    with tile.TileContext(nc) as tc, ExitStack() as ctx:
        const = ctx.enter_context(tc.tile_pool(name="const", bufs=1))
        act = ctx.enter_context(tc.tile_pool(name="act", bufs=2))
        small = ctx.enter_context(tc.tile_pool(name="small", bufs=2))
        psum = ctx.enter_context(tc.tile_pool(name="psum", bufs=1, space="PSUM"))

        def load_inputs(b):
            ins = {}
            for nm, ap in (("q", qT), ("k", kT), ("v", vT)):
                # v is consumed by the last projection tasks of a stage, so a
                # single buffer stalls nothing while saving 8KB/partition
                t = act.tile([128, DT, S], bf16, name=f"in_{nm}", tag=f"in_{nm}",
                             bufs=1 if nm == "v" else IN_BUFS)
                nc.sync.dma_start(
                    t[:, :, :], ap[b].rearrange("(t p) s -> p t s", p=128))
                ins[nm] = t
            return ins

        # ---- pair-0 inputs first so projections can start immediately --
        pending_ins = {0: load_inputs(0)}

        # ---- constants (wq/wk/wv early, wo last) -----------------------
        w_tiles = {}
        for wname, wap in (("wq", wq), ("wk", wk), ("wv", wv), ("wo", wo)):
            t = const.tile([128, DT, D], bf16, name=wname, tag=wname)
            if wname != "wo":
                nc.sync.dma_start(
                    t[:, :, :], wap.rearrange("(t p) o -> p t o", p=128))
            w_tiles[wname] = t
        if b_loc > 1:
            pending_ins[1] = load_inputs(1)
        wg_sb = const.tile([128, DT, H_DYN], bf16, name="wg_sb", tag="wg_sb")
        nc.sync.dma_start(wg_sb[:, :, :], wg.rearrange("(t p) j -> p t j", p=128))
        bq_sb = const.tile([128, DT], f32, name="bq_sb", tag="bq_sb")
        nc.sync.dma_start(bq_sb[:], bqt[:])
        bk_sb = const.tile([128, DT], f32, name="bk_sb", tag="bk_sb")
        nc.sync.dma_start(bk_sb[:], bkt[:])
        ident_sb = const.tile([128, 128], bf16, name="ident_sb", tag="ident_sb")
        nc.sync.dma_start(ident_sb[:], ident[:])
        if has_bvo:
            bv_sb = const.tile([1, D], f32, name="bv_sb", tag="bv_sb")
            nc.sync.dma_start(bv_sb[:], bvb[:])
            bo_sb = const.tile([1, D], f32, name="bo_sb", tag="bo_sb")
            nc.sync.dma_start(bo_sb[:], bob[:])
            bvb_sb = const.tile([128, D], f32, name="bvb_sb", tag="bvb_sb")
            nc.gpsimd.partition_broadcast(bvb_sb[:], bv_sb[:])
            bob_sb = const.tile([128, D], f32, name="bob_sb", tag="bob_sb")
            nc.gpsimd.partition_broadcast(bob_sb[:], bo_sb[:])

        mask_tiles = []
        for u in range(uniq_n):
            t = const.tile([128, 128], bf16, name=f"mask{u}", tag=f"mask{u}")
            nc.sync.dma_start(t[:], maskt[u])
            mask_tiles.append(t)
        if need_allneg:
            allneg = const.tile([128, 128], bf16, name="allneg", tag="allneg")
            nc.vector.memset(allneg[:], NEG)

        ones_bf = const.tile([128, 1], bf16, name="ones_bf", tag="ones_bf")
        nc.vector.memset(ones_bf[:], 1.0)
        nc.sync.dma_start(
            w_tiles["wo"][:, :, :], wo.rearrange("(t p) o -> p t o", p=128))

        # ---------------------------------------------------------------
        def proj_tasks(b, ins, st):
            """Return a list of closures emitting the projection matmul
            groups + routing for batch b; st is this batch's state dict."""
            tasks = []

            def qk_group(dst, src, wn, bias, eng, t):
                def run():
                    ps = psum.tile([128, S], f32, name="mm_ps", tag="mm",
                                   bufs=MM_BUFS)
                    for d in range(DT):
                        nc.tensor.matmul(
                            ps[:],
                            w_tiles[wn][:, d, t * 128:(t + 1) * 128],
                            src[:, d, :],
                            start=(d == 0),
                            stop=(d == DT - 1),
                        )
                    if eng == "act":
                        nc.scalar.activation(
                            dst[:, t, :], ps[:], AF.Identity,
                            bias=bias[:, t:t + 1],
                        )
                    else:
                        nc.vector.tensor_scalar_add(
                            dst[:, t, :], ps[:], bias[:, t:t + 1]
                        )
                return run

            for t in range(DT):
                tasks.append(qk_group(st["qpT"], ins["q"], "wq", bq_sb, "act", t))
            for t in range(DT):
                tasks.append(qk_group(st["kpT"], ins["k"], "wk", bk_sb, "dve", t))

            def vp_group(sb, c):
                def run():
                    vp = st["vp"]
                    ps = psum.tile([128, S], f32, name="mm_ps", tag="mm",
                                   bufs=MM_BUFS)
                    for d in range(DT):
                        nc.tensor.matmul(
                            ps[:],
                            ins["v"][:, d, sb * 128:(sb + 1) * 128],
                            w_tiles["wv"][:, d, c * 512:(c + 1) * 512],
                            start=(d == 0),
                            stop=(d == DT - 1),
                        )
                    src2 = ps[:].rearrange("p (h e) -> p h e", e=DK)
                    dst2 = vp[:, sb, c * 8:(c + 1) * 8, 0:DK]
                    if has_bvo:
                        nc.vector.scalar_tensor_tensor(
                            dst2, src2, 1.0,
                            bvb_sb[:, c * 512:(c + 1) * 512].rearrange(
                                "p (h e) -> p h e", e=DK),
                            op0=ALU.mult, op1=ALU.add,
                        )
                    else:
                        nc.vector.tensor_copy(dst2, src2)
                return run

            for sb in range(SB):
                for c in range(2):
                    tasks.append(vp_group(sb, c))

            def routing_task():
                qpT = st["qpT"]
                # one PSUM bank (shared with the score tiles' rotation):
                # cols 0:48 = gate logits (4 sb x 12), cols 48:60 = routing
                # mean accumulator ([1,12] region)
                ps_g = psum.tile([128, 512], f32, name="ps_g", tag="st",
                                 bufs=ST_BUFS)
                for sb in range(SB):
                    for t in range(DT):
                        nc.tensor.matmul(
                            ps_g[:, sb * H_DYN:(sb + 1) * H_DYN],
                            qpT[:, t, sb * 128:(sb + 1) * 128],
                            wg_sb[:, t, :],
                            start=(sb == 0 and t == 0),
                            stop=False,
                            skip_group_check=True,
                        )
                gexp = small.tile([128, SB, H_DYN], f32, name="gexp", tag="gexp")
                nc.scalar.activation(
                    gexp[:], ps_g[:, 0:SB * H_DYN].rearrange(
                        "p (s j) -> p s j", j=H_DYN),
                    AF.Exp)
                gsum = small.tile([128, SB], f32, name="gsum", tag="gsum")
                nc.vector.tensor_reduce(
                    gsum[:], gexp[:], axis=mybir.AxisListType.X, op=ALU.add)
                ginv = small.tile([128, SB], f32, name="ginv", tag="ginv")
                nc.vector.reciprocal_approx_fast(ginv[:], gsum[:])
                m1 = small.tile([128, SB], f32, name="m1", tag="m1")
                nc.vector.tensor_reduce(
                    m1[:], gexp[:], axis=mybir.AxisListType.X, op=ALU.max)
                g2 = small.tile([128, SB, H_DYN], f32, name="g2", tag="g2")
                m2 = small.tile([128, SB], f32, name="m2", tag="m2")
                sel = small.tile([128, SB, H_DYN], f32, name="sel", tag="sel")
                for sb in range(SB):
                    # knock out the top-1, re-max for top-2 threshold
                    eqm = small.tile([128, H_DYN], f32, name="eqm", tag="eqm")
                    nc.vector.tensor_scalar(
                        eqm[:], gexp[:, sb, :], m1[:, sb:sb + 1], None,
                        op0=ALU.is_equal)
                    nc.vector.scalar_tensor_tensor(
                        g2[:, sb, :], eqm[:], NEG, gexp[:, sb, :],
                        op0=ALU.mult, op1=ALU.add)
                nc.vector.tensor_reduce(
                    m2[:], g2[:], axis=mybir.AxisListType.X, op=ALU.max)
                for sb in range(SB):
                    nc.vector.tensor_scalar(
                        sel[:, sb, :], gexp[:, sb, :], m2[:, sb:sb + 1], None,
                        op0=ALU.is_ge)
                gdyn = small.tile([128, SB, H_DYN], f32, name="gdyn", tag="gdyn")
                nc.vector.tensor_tensor(gdyn[:], gexp[:], sel[:], op=ALU.mult)
                for sb in range(SB):
                    nc.tensor.matmul(
                        ps_g[0:1, 48:48 + H_DYN],
                        ginv[:, sb:sb + 1],
                        gdyn[:, sb, :],
                        start=False, stop=(sb == SB - 1),
                        skip_group_check=True,
                    )
                routing_sb = small.tile([1, H], f32, name="routing_sb",
                                        tag="routing_sb")
                nc.vector.memset(routing_sb[0:1, 0:H_SH], 1.0)
                nc.scalar.mul(routing_sb[0:1, H_SH:H], ps_g[0:1, 48:48 + H_DYN],
                              1.0 / S)
                routing_bc = small.tile([128, H], f32, name="routing_bc",
                                        tag="routing_bc")
                nc.gpsimd.partition_broadcast(routing_bc[:], routing_sb[:])
                st["routing"] = routing_bc

            tasks.append(routing_task)
            return tasks

        def scores_phase(st, h):
            """Emit prefills + score matmuls + exp for head h; return the
            deferred ctx matmul operand list."""
            qpT, kpT, vp = st["qpT"], st["kpT"], st["vp"]
            se = act.tile([128, se_cols], bf16, name="se", tag="se",
                          bufs=SE_BUFS)
            ctx_mms = []
            ph = (h % 2) * 64
            th = h // 2
            for grp in packs:
                tot = grp[-1][3] + grp[-1][2]
                ps_st = psum.tile([128, 512], f32, name="ps_st", tag="st",
                                  bufs=ST_BUFS)
                first = True
                for gi, (kb, q0, n, off, sc) in enumerate(grp):
                    if MASKMODE == "prefill":
                        # additive-mask prefill: tiny identity matmuls drop
                        # the mask into PSUM; the score matmul accumulates
                        # on top.
                        for qb in range(q0 // 128, SB):
                            mi = mixed.get((qb, kb))
                            if mi is None:
                                continue
                            col = off + qb * 128 - q0
                            src = mask_tiles[mi] if mi >= 0 else allneg
                            nc.tensor.matmul(
                                ps_st[:, col:col + 128], ident_sb[:], src[:],
                                start=first, stop=False, skip_group_check=True,
                            )
                            first = False
                    # score matmul, optionally split at prefilled/pristine
                    # block boundaries (uniform PSUM regions for CoreSim)
                    if SPLIT and MASKMODE == "prefill":
                        runs = []
                        for qb in range(q0 // 128, SB):
                            pref = (qb, kb) in mixed
                            if runs and runs[-1][2] == pref:
                                runs[-1][1] = (qb + 1) * 128
                            else:
                                runs.append([qb * 128, (qb + 1) * 128, pref])
                    else:
                        runs = [[q0, S, False]]
                    last_entry = gi == len(grp) - 1
                    for ri, (rs, re, pref) in enumerate(runs):
                        nc.tensor.matmul(
                            ps_st[:, off + rs - q0:off + re - q0],
                            kpT[ph:ph + 64, th, kb * 128:(kb + 1) * 128],
                            qpT[ph:ph + 64, th, rs:re],
                            start=first,
                            stop=(last_entry and ri == len(runs) - 1),
                            skip_group_check=True,
                        )
                        first = False
                    ctx_mms.append((kb, sc, q0, n))
                sc0 = grp[0][4]
                nc.scalar.activation(
                    se[:, sc0:sc0 + tot], ps_st[:, 0:tot], AF.Exp,
                    scale=1.0 / np.sqrt(DK))
                if MASKMODE == "affine":
                    # zero se where q_local < k_local on the transposed
                    # diagonal blocks (iota = q_local - k_local)
                    for (kb, q0, n, off, sc) in grp:
                        for qb in range(q0 // 128, SB):
                            if (qb, kb) in mixed:
                                col = sc + qb * 128 - q0
                                nc.gpsimd.affine_select(
                                    se[:, col:col + 128],
                                    se[:, col:col + 128],
                                    pattern=[[1, 128]],
                                    compare_op=ALU.is_ge,
                                    fill=0.0,
                                    base=0,
                                    channel_multiplier=-1,
                                )
            if DEBUG and dbg_flag.get("se"):
                dbg_flag["se"] = False
                nc.sync.dma_start(dbg["d_se"][:, 0:se_cols], se[:])
            return se, ctx_mms

        def ctx_phase(st, h, se, ctx_mms):
            vp, ctxT = st["vp"], st["ctxT"]
            ph = (h % 2) * 64
            th = h // 2
            ps_ctx = psum.tile([DK + 1, S], f32, name="ps_ctx", tag="ctx",
                               bufs=CTX_BUFS)
            # ascending q0 so the first (widest) matmul covers all later
            # ones' columns — keeps each PSUM region uniformly pending.
            # Column q=0 is handled purely by ones-matmuls (the reference
            # zeroes score row 0 for ALL k, making exp()=1 everywhere), so
            # q0==0 entries skip their first column and no se[:,0] fixup
            # memset is needed.
            ctx_mms.sort(key=lambda m: m[2])
            mms = []
            for (kb, sc, q0, n) in ctx_mms:
                skip = 1 if q0 == 0 else 0
                mms.append((vp[:, kb, h, :],
                            se[:, sc + skip:sc + n],
                            ps_ctx[:, q0 + skip:q0 + n]))
            for kb in range(SB):
                mms.append((vp[:, kb, h, :], ones_bf[:], ps_ctx[:, 0:1]))
            for i, (lhsT, rhs, dst) in enumerate(mms):
                nc.tensor.matmul(
                    dst, lhsT, rhs,
                    start=(i == 0), stop=(i == len(mms) - 1),
                    skip_group_check=True,
                )

            # denominator row -> SBUF via ACT (a standard op with correct
            # PSUM-drain hazard handling; the custom-DVE reciprocal reading
            # PSUM directly raced the PE writeback on HW)
            drow = small.tile([1, S], f32, name="drow", tag="drow", bufs=2)
            nc.scalar.copy(drow[:], ps_ctx[DK:DK + 1, :])
            bc = small.tile([64, S], f32, name="bc", tag="bc", bufs=2)
            if NORM == "divide":
                nc.gpsimd.partition_broadcast(bc[:], drow[:], channels=64)
                op1 = ALU.divide
            else:
                recip = small.tile([1, S], f32, name="recip", tag="recip",
                                   bufs=2)
                nc.vector.reciprocal_approx_fast(recip[:], drow[:])
                nc.gpsimd.partition_broadcast(bc[:], recip[:], channels=64)
                op1 = ALU.mult
            if DEBUG and dbg_flag.get("bc"):
                dbg_flag["bc"] = False
                dps = small.tile([DK + 1, S], f32, name="dps", tag="dps")
                nc.scalar.copy(dps[:], ps_ctx[:])
                nc.sync.dma_start(dbg["d_ctxps"][:], dps[:])
                nc.sync.dma_start(dbg["d_recip"][:], drow[:])
                nc.sync.dma_start(dbg["d_bc"][:], bc[:])
            nc.vector.scalar_tensor_tensor(
                ctxT[ph:ph + 64, th, :],
                ps_ctx[0:DK, :],
                st["routing"][0:64, h:h + 1],
                bc[:],
                op0=ALU.mult, op1=op1,
            )

        def outproj_tasks(st, b):
            ctxT = st["ctxT"]

            def group(sb, c):
                def run():
                    ps = psum.tile([128, S], f32, name="mm_ps", tag="mm",
                                   bufs=MM_BUFS)
                    for t in range(DT):
                        nc.tensor.matmul(
                            ps[:],
                            ctxT[:, t, sb * 128:(sb + 1) * 128],
                            w_tiles["wo"][:, t, c * 512:(c + 1) * 512],
                            start=(t == 0),
                            stop=(t == DT - 1),
                        )
                    ob = small.tile([128, S], f32, name="ob", tag="ob", bufs=2)
                    if has_bvo:
                        nc.vector.scalar_tensor_tensor(
                            ob[:], ps[:], 1.0, bob_sb[:, c * 512:(c + 1) * 512],
                            op0=ALU.mult, op1=ALU.add,
                        )
                    else:
                        nc.scalar.copy(ob[:], ps[:])
                    nc.sync.dma_start(
                        out[b, sb * 128:(sb + 1) * 128, c * 512:(c + 1) * 512],
                        ob[:],
                    )
                return run

            return [group(sb, c) for sb in range(SB) for c in range(2)]

        def merge_tasks(a, bl):
            """Interleave two task lists by fractional position."""
            keyed = [((j + 0.5) / max(len(a), 1), t) for j, t in enumerate(a)]
            keyed += [((j + 0.5) / max(len(bl), 1), t) for j, t in enumerate(bl)]
            keyed.sort(key=lambda kt: kt[0])
            return [t for _, t in keyed]

        def new_state():
            return {
                "qpT": act.tile([128, DT, S], bf16, name="qpT", tag="qpT"),
                "kpT": act.tile([128, DT, S], bf16, name="kpT", tag="kpT"),
                "vp": None,
                "ctxT": act.tile([128, DT, S], bf16, name="ctxT", tag="ctxT"),
                "routing": None,
            }

        def make_vp(st):
            vp = act.tile([128, SB, H, DK + 1], bf16, name="vp", tag="vp")
            nc.vector.memset(vp[:, :, :, DK:DK + 1], 1.0)
            st["vp"] = vp

        # ---- software pipeline: attn(b) || proj(b+1) -------------------
        order = [bb for _ in range(repeat) for bb in range(b_loc)]
        first = True
        dbg_flag = {"se": DEBUG, "bc": DEBUG}

        # prologue: projections of the first batch
        st_cur = new_state()
        make_vp(st_cur)
        ins0 = pending_ins.pop(order[0], None) or load_inputs(order[0])
        for t in proj_tasks(order[0], ins0, st_cur):
            t()
        if DEBUG:
            nc.sync.dma_start(dbg["d_qpT"][:, :, :], st_cur["qpT"][:, :, :])
            nc.sync.dma_start(dbg["d_kpT"][:, :, :], st_cur["kpT"][:, :, :])
            nc.sync.dma_start(dbg["d_vp"][:, :, :, :], st_cur["vp"][:, :, :, :])
            nc.sync.dma_start(dbg["d_routing"][:], st_cur["routing"][:])

        st_prev = None
        b_prev = None
        for i, b in enumerate(order):
            nxt = order[i + 1] if i + 1 < len(order) else None
            tasks = []
            st_nxt = None
            if nxt is not None:
                st_nxt = new_state()
                make_vp(st_nxt)
                if first and nxt in pending_ins:
                    ins = pending_ins.pop(nxt)
                else:
                    ins = load_inputs(nxt)
                tasks = proj_tasks(nxt, ins, st_nxt)
            first = False
            # out-projection of the PREVIOUS batch rides in this stage's
            # task interleave so it never waits on the last heads' chains
            if st_prev is not None:
                tasks = merge_tasks(outproj_tasks(st_prev, b_prev), tasks)
            ntasks = len(tasks)
            done = 0
            pend = []
            for h in range(H):
                se_h, cm = scores_phase(st_cur, h)
                if len(pend) >= CTXDELAY:
                    ctx_phase(st_cur, *pend.pop(0))
                pend.append((h, se_h, cm))
                want = (h + 1) * ntasks // H
                while done < want:
                    tasks[done]()
                    done += 1
            for p in pend:
                ctx_phase(st_cur, *p)
            if DEBUG and i == 0:
                nc.sync.dma_start(dbg["d_ctxT"][:, :, :], st_cur["ctxT"][:, :, :])
            st_prev, b_prev = st_cur, b
            st_cur = st_nxt
        for t in outproj_tasks(st_prev, b_prev):
            t()

    nc.compile()
    return nc


def _host_prep(inputs):
    """Host-side input prep shared by kernel() and the timing harness.

    Returns (shared, qT, kT, vT, cache_key_parts).
    """
    q = np.asarray(inputs["q"])
    k = np.asarray(inputs["k"])
    v = np.asarray(inputs["v"])
    mask = np.asarray(inputs["mask"]).reshape(S, S)
    Wq, bq = np.asarray(inputs["Wq"]), np.asarray(inputs["bq"])
    Wk, bk = np.asarray(inputs["Wk"]), np.asarray(inputs["bk"])
    Wv, bv = np.asarray(inputs["Wv"]), np.asarray(inputs["bv"])
    Wg = np.asarray(inputs["Wg"])
    Wo, bo = np.asarray(inputs["Wo"]), np.asarray(inputs["bo"])

    bf = ml_dtypes.bfloat16
    qs, mixed, uniq, causal = _classify_mask(mask)
    has_bvo = bool(np.any(bv) or np.any(bo))

    qT = np.ascontiguousarray(q.astype(bf).transpose(0, 2, 1))
    kT = np.ascontiguousarray(k.astype(bf).transpose(0, 2, 1))
    vT = np.ascontiguousarray(v.astype(bf).transpose(0, 2, 1))

    shared = {
        "wq": Wq.astype(bf), "wk": Wk.astype(bf), "wv": Wv.astype(bf),
        "wo": Wo.astype(bf), "wg": Wg.astype(bf),
        "bqt": np.ascontiguousarray(
            bq.astype(np.float32).reshape(DT, 128).T),
        "bkt": np.ascontiguousarray(
            bk.astype(np.float32).reshape(DT, 128).T),
        "ident": np.eye(128, dtype=bf),
    }
    if has_bvo:
        shared["bvb"] = bv.astype(np.float32).reshape(1, D)
        shared["bob"] = bo.astype(np.float32).reshape(1, D)
    if uniq:
        shared["maskt"] = np.stack(uniq, axis=0).astype(bf)
    return shared, qT, kT, vT, (mask, qs, mixed, uniq, causal, has_bvo)


def kernel(**inputs):
    shared, qT, kT, vT, (mask, qs, mixed, uniq, causal, has_bvo) = \
        _host_prep(inputs)

    cache_key = ("v3", mask.tobytes(), has_bvo)
    if cache_key not in _CACHE:
        _CACHE[cache_key] = _build(qs, mixed, len(uniq), causal, has_bvo=has_bvo)
    nc = _CACHE[cache_key]

    in_maps = []
    for c in range(N_CORES):
        sl = slice(c * B_LOC, (c + 1) * B_LOC)
        m = dict(shared)
        m["qT"] = qT[sl]
        m["kT"] = kT[sl]
        m["vT"] = vT[sl]
        in_maps.append(m)

    from concourse.bass_utils import run_bass_kernel_spmd

    kw = {}
    if PROFILE:
        import tempfile
        kw = dict(trace=True, tmpdir=tempfile.mkdtemp(prefix="moh_trace_"))
    res = None
    last_exc = None
    for _attempt in range(3):
        try:
            res = run_bass_kernel_spmd(
                nc, in_maps, core_ids=list(range(N_CORES)), **kw)
            break
        except Exception as e:  # transient axon/NRT device errors
            last_exc = e
    if res is None:
        raise last_exc
    LAST["exec_time_ns"] = res.exec_time_ns
    LAST["profile_json"] = res.profile_json
    if PROFILE:
        LAST["tmpdir"] = kw.get("tmpdir")
    outs = [res.results[c]["out"] for c in range(N_CORES)]
    return np.concatenate(outs, axis=0).astype(np.float32)
